# revision 37
# baseline (speedup 1.0000x reference)
"""Multi-head attention (B=2, N=2048, M=1024, H=16) on 8 trn2 NeuronCores.

Sharding: core c handles batch b = c//4 and heads 4*(c%4) .. 4*(c%4)+4.
Each core computes its 4 heads' attention and a partial output projection;
the host sums the 4 partials per batch and adds the constant bias term
(bo + bv @ Wo.T - exact because softmax rows sum to 1).

Design (fp16 compute, f32 PSUM accumulation):
  - Steady state is jointly DVE/ACT-paced (~39 us per 512-query block):
    mask-multiply (PSUM-f32 source, 1x DVE mode) and softmax exp (ACT) are
    the structural floors; everything else is scheduled into their slack.
  - attnV matmuls are software-pipelined one kc2 block behind their exp
    (pend_av) so each PE instruction's dependencies resolve a block early.
  - Minimal prefix (K block 0 + Q block 0 via token-quartered xt DMAs,
    prioritized DMA order); all remaining K/V/Q projections drip through
    per-kc2 hooks. Held PSUM accumulators obey the tag-rotation rule:
    last allocation of their first hook, first emission of the next.
  - collect drains accs into o_collb [65,4,512] f16 (row 64 = softmax
    denominators); rearrange DMAs (ds, o_pairs) issue at collect time so
    the norm chain (reciprocal -> r0 -> zero-stride broadcast DMAs -> 2x
    f16 multiply) never parks an engine stream on DMA latency.
  - O-projection chunks spread one per hook; next q4's mask prefetched
    mid-pair-0; dummy matmuls on a memset tile hold the PE p-state ramp
    through DMA waits and the final norm lull; the tail O-projection
    batches FD-1024 drains through the freed sct PSUM banks.
"""
import sys
import os

sys.path.insert(0, '/opt/trn_rl_repo')

import numpy as np
import ml_dtypes

import concourse.bass as bass
import concourse.tile as tile
from concourse import mybir
from concourse.vector_clock import ScopedClock
from concourse.bass_utils import run_bass_kernel_spmd

dt = mybir.dt
F32, F16 = dt.float32, dt.float16
AF = mybir.ActivationFunctionType
OP = mybir.AluOpType

B, N, M, H = 2, 2048, 1024, 16
DK = M // H            # 64
HPC = 4                # heads per core
HD = HPC * DK          # 256 head dims per core
NCORES = 8
QC = 4                 # query blocks of 512
KC = 16                # key chunks of 128
MC = 8                 # model-dim chunks of 128
SC = 16                # seq chunks of 128

LAST_RESULTS = None


class TC(tile.TileContext):
    """TileContext patched for a walrus build that only accepts ONE sync-wait
    per instruction: excess waits are peeled onto same-engine NoOps inserted
    immediately before the instruction (engine streams are in-order, so the
    waits still gate the instruction exactly as before)."""
    MAXW = 1

    def _split_waits(self, inst):
        si = inst.sync_info
        if si is None or si.on_wait is None or len(si.on_wait) <= self.MAXW:
            return
        if inst.engine == mybir.EngineType.Unassigned:
            return
        waits = list(si.on_wait)
        for w in waits[:-self.MAXW]:
            nop = mybir.InstNoOp(name=f"nopw-{self.nc.next_id()}", ins=[], outs=[])
            nop.engine = inst.engine
            nop.sync_info = mybir.SyncInfo(on_wait=[w], on_update=[])
            super()._add_instruction(nop)
        si.on_wait = waits[-self.MAXW:]
        inst.sync_info = si

    def _add_instruction(self, inst):
        self._split_waits(inst)
        super()._add_instruction(inst)

    def _drain_and_barrier(self, tick_clock, wait_clock):
        drain_inst = self.nc.sync.drain()
        wait_clock.add_sem_waits(drain_inst.ins,
                                 ScopedClock({None: tick_clock.global_clock}))
        si = drain_inst.ins.sync_info
        if si is not None and si.on_wait is not None and len(si.on_wait) > 1:
            waits = list(si.on_wait)
            si.on_wait = waits[:1]
            drain_inst.ins.sync_info = si
            for w in waits[1:]:
                nop = self.nc.sync.nop(nofuse=True)
                nop.ins.sync_info = mybir.SyncInfo(on_wait=[w], on_update=[])
        self.nc.all_engine_barrier()
        assert self.sems is not None
        popped = self.nc._tile_sem_poison_stack.pop()
        assert popped is self._sem_poison
        self.nc.clear_and_free_semaphores(list(self.sems.allocated().values()))
        self.nc.all_engine_barrier()


def _bcast_mid(ap, n):
    """[P, F] AP -> [P, n, F] AP with a zero-stride middle dim."""
    layout = list(ap.ap)
    assert len(layout) == 2
    new_layout = [layout[0], [0, n], layout[1]]
    return bass.AP(ap.tensor, ap.offset, new_layout)


def _part_bcast(ap, n):
    """[1, F] AP -> [1, n, F] AP replicating the partition's data n times
    via a zero-stride middle dim (DMA src; partition dim keeps step 1)."""
    layout = list(ap.ap)
    assert len(layout) == 2 and layout[0][1] == 1
    new_layout = [layout[0], [0, n], layout[1]]
    return bass.AP(ap.tensor, ap.offset, new_layout)


def _build_program(repeat=1):
    nc = bass.Bass(num_devices=NCORES)

    xT = nc.dram_tensor("xT", [M, N], F16, kind="ExternalInput")
    mask4 = nc.dram_tensor("mask4", [QC, 128, KC, 512], F16, kind="ExternalInput")
    wq = nc.dram_tensor("wq", [M, HD], F16, kind="ExternalInput")   # Wq[slice].T
    wk = nc.dram_tensor("wk", [M, HD], F16, kind="ExternalInput")
    wv = nc.dram_tensor("wv", [M, HD], F16, kind="ExternalInput")
    wo2 = nc.dram_tensor("wo2", [2, 128, M], F16, kind="ExternalInput")  # pair rows
    bq2 = nc.dram_tensor("bq2", [128, 2], F32, kind="ExternalInput")  # bq[slice]/8
    bk2 = nc.dram_tensor("bk2", [128, 2], F32, kind="ExternalInput")
    partial = nc.dram_tensor("partial", [N, M], F16, kind="ExternalOutput")

    with TC(nc) as tc:
      for _rep in range(repeat):
        with tc.tile_pool(name="persist", bufs=1) as pp:
            # ---- persistent tiles ----
            wo_t = [pp.tile([128, M], F16, tag=f"wo{p}", name=f"wo_t{p}")
                    for p in range(2)]
            bq_t = pp.tile([128, 2], F32)
            bk_t = pp.tile([128, 2], F32)

            qT_sb = [pp.tile([128, N], F16, tag=f"qT{pt}", name=f"qT_sb{pt}")
                     for pt in range(2)]
            kT_sb = [pp.tile([128, N], F16, tag=f"kT{pt}", name=f"kT_sb{pt}")
                     for pt in range(2)]
            v_aug = pp.tile([128, SC, HPC, DK + 1], F16)
            warm = pp.tile([128, 512], F16)
            nc.gpsimd.memset(warm[:], 0.0)
            nc.gpsimd.memset(v_aug[:], 1.0)

            # ---- projection inputs ----
            mp_ctx = tc.tile_pool(name="maskp", bufs=2)
            mp = mp_ctx.__enter__()
            xp_ctx = tc.tile_pool(name="projp", bufs=1)
            xp = xp_ctx.__enter__()
            pj_ctx = tc.tile_pool(name="pjps", bufs=1, space="PSUM")
            pj = pj_ctx.__enter__()

            # Startup DMAs in dependency-priority order: the sim (and HW
            # aggregate bandwidth) serializes transfers, so the order below
            # IS the arrival schedule. Split across both HWDGE queues so
            # issue overhead overlaps.
            xt = xp.tile([128, MC, N], F16)
            xt_r = xT.rearrange("(c p) n -> p c n", p=128)
            wk_t = xp.tile([128, MC, HD], F16)
            wq_t = xp.tile([128, MC, HD], F16)
            wv_t = xp.tile([128, MC, HD], F16)
            mw0 = mp.tile([128, KC, 512], F16, tag="mask", name="mw_0")
            nc.sync.dma_start(wk_t[:], wk.rearrange("(c p) h -> p c h", p=128))
            nc.sync.dma_start(bq_t[:], bq2[:])
            nc.sync.dma_start(bk_t[:], bk2[:])
            nc.sync.dma_start(xt[:, :, 0:512], xt_r[:, :, 0:512])
            nc.sync.dma_start(wq_t[:], wq.rearrange("(c p) h -> p c h", p=128))
            nc.sync.dma_start(mw0[:, 0:8, :], mask4[0, :, 0:8, :])
            nc.sync.dma_start(wv_t[:], wv.rearrange("(c p) h -> p c h", p=128))
            for c in range(1, 3):
                nc.sync.dma_start(xt[:, :, 512 * c:512 * (c + 1)],
                                  xt_r[:, :, 512 * c:512 * (c + 1)])
            nc.sync.dma_start(mw0[:, 8:16, :], mask4[0, :, 8:16, :])
            nc.sync.dma_start(xt[:, :, 1536:2048], xt_r[:, :, 1536:2048])
            for p in range(2):
                nc.sync.dma_start(wo_t[p][:], wo2[p])

            # PE p-state prewarm: dummy matmuls on the memset tile keep the
            # ramp clock running through the initial DMA wait.
            pwacc = pj.tile([128, 512], F32, tag="pj3", name="pwacc")
            for _ in range(9):
                nc.tensor.matmul(pwacc[:], warm[:, 0:128], warm[:],
                                 start=True, stop=True)

            def emit_k_block(pt, q4, acck, epilogue=True, quarters=range(4)):
                qs = slice(q4 * 512, (q4 + 1) * 512)
                for qtr in quarters:
                    for mc in (2 * qtr, 2 * qtr + 1):
                        nc.tensor.matmul(acck[:],
                                         wk_t[:, mc, pt * 128:(pt + 1) * 128],
                                         xt[:, mc, qs],
                                         start=(mc == 0), stop=(mc == MC - 1))
                if epilogue:
                    nc.scalar.activation(kT_sb[pt][:, qs], acck[:],
                                         AF.Identity, bias=bk_t[:, pt:pt + 1],
                                         scale=1.0)

            def emit_q_half(pt, q4, st, half, pool, tag):
                qs = slice(q4 * 512, (q4 + 1) * 512)
                if half == 0:
                    st['accq'] = pool.tile([128, 512], F32, tag=tag,
                                           name=f"accq_{pt}_{q4}")
                accq = st['accq']
                for mc in range(half * MC // 2, (half + 1) * MC // 2):
                    nc.tensor.matmul(accq[:],
                                     wq_t[:, mc, pt * 128:(pt + 1) * 128],
                                     xt[:, mc, qs],
                                     start=(mc == 0), stop=(mc == MC - 1))
                if half == 1:
                    nc.scalar.activation(qT_sb[pt][:, qs], accq[:],
                                         AF.Identity, bias=bq_t[:, pt:pt + 1],
                                         scale=0.125)

            def emit_q(pt, q4, pool, tag):
                st = {}
                emit_q_half(pt, q4, st, 0, pool, tag)
                emit_q_half(pt, q4, st, 1, pool, tag)

            def emit_v(sc, pool, tag):
                accv = pool.tile([128, 512], F32, tag=tag)
                for mc in range(MC):
                    nc.tensor.matmul(accv[:, 0:HD],
                                     xt[:, mc, sc * 128:(sc + 1) * 128],
                                     wv_t[:, mc, :],
                                     start=(mc == 0), stop=(mc == MC - 1))
                nc.vector.tensor_copy(v_aug[:, sc, :, 0:DK], accv[:, 0:HD])

            # Minimal prefix: K-pt0 (xt arrives in quarters, so interleave the
            # quarter chunks across the four key blocks), Q0-pt0, then the
            # first pair-1 requirements (K-pt1 block 0, Q0-pt1) and V0/V1.
            acck00 = pj.tile([128, 512], F32, tag="pj0", name="acck00")
            emit_k_block(0, 0, acck00)
            emit_q(0, 0, pj, "pj1")
            pj_ctx.__exit__(None, None, None)

            # ---- attention ----
            sw_ctx = tc.tile_pool(name="sbwork", bufs=3)
            sw = sw_ctx.__enter__()
            np_ctx = tc.tile_pool(name="normp", bufs=2)
            np_ = np_ctx.__enter__()
            op_ctx = tc.tile_pool(name="outp", bufs=3)
            op_ = op_ctx.__enter__()
            ps_ctx = tc.tile_pool(name="pssct", bufs=2, space="PSUM")
            psw = ps_ctx.__enter__()
            pa_ctx = tc.tile_pool(name="psacc", bufs=1, space="PSUM")
            psa = pa_ctx.__enter__()
            po_ctx = tc.tile_pool(name="psout", bufs=2, space="PSUM")
            pso = po_ctx.__enter__()

            def emit_pair(q4, pair, mw, hooks=None, defer_av=False):
                """scores -> mask-mul -> exp -> attnV accumulate for 2 heads.
                hooks[kc2] is a list of closures emitted before that kc2
                block, spreading other work into the steady-state slack.
                defer_av shifts each kc2's attnV matmuls one block later so
                hook-emitted v_aug chunks may land inside this pair without
                deadlocking the in-order PE stream."""
                qs = slice(q4 * 512, (q4 + 1) * 512)
                accs = psa.tile([65, 2, 512], F32, tag="accs",
                                name=f"accs_{q4}_{pair}")

                def emit_av(kc2, pe):
                    for kci in range(2):
                        kc = kc2 * 2 + kci
                        for hh in range(2):
                            nc.tensor.matmul(accs[:, hh, :],
                                             v_aug[:, kc, 2 * pair + hh, :],
                                             pe[:, kci, hh, :],
                                             start=(kc == 0), stop=(kc == KC - 1))

                pend_av = None
                for kc2 in range(KC // 2):
                    if hooks and kc2 in hooks:
                        for h in hooks[kc2]:
                            h()
                    pm = sw.tile([128, 2, 2, 512], F16, tag="pm",
                                 name=f"pm_{q4}_{pair}_{kc2}")
                    for kci in range(2):
                        kc = kc2 * 2 + kci
                        ks = slice(kc * 128, (kc + 1) * 128)
                        sct = psw.tile([128, 2, 512], F32, tag="sct",
                                       name=f"sct_{q4}_{pair}_{kc}")
                        nc.tensor.matmul(sct[:, 0, :],
                                         kT_sb[pair][0:64, ks],
                                         qT_sb[pair][0:64, qs],
                                         start=True, stop=True)
                        nc.tensor.matmul(sct[:, 1, :],
                                         kT_sb[pair][64:128, ks],
                                         qT_sb[pair][64:128, qs],
                                         start=True, stop=True)
                        nc.vector.tensor_tensor(pm[:, kci], sct[:],
                                                _bcast_mid(mw[:, kc, :], 2),
                                                op=OP.mult)
                    if pend_av is not None:
                        emit_av(*pend_av)
                        pend_av = None
                    pe = sw.tile([128, 2, 2, 512], F16, tag="pe",
                                 name=f"pe_{q4}_{pair}_{kc2}")
                    nc.scalar.activation(pe[:], pm[:], AF.Exp)
                    pend_av = (kc2, pe)
                if pend_av is not None:
                    emit_av(*pend_av)
                return accs

            def emit_collect(q4, pair, accs, o_collb, o_pairs, ds):
                """Drain the pair's accumulators (frees accs) and issue the
                rearrange DMAs now so the norm chain finds them resolved."""
                for hh in range(2):
                    nc.scalar.copy(o_collb[:, 2 * pair + hh, :],
                                   accs[:, hh, :])
                nc.sync.dma_start(ds[32 * pair:32 * pair + 16, :],
                                  o_collb[64:65, 2 * pair:2 * pair + 2, :])
                for hh in range(2):
                    nc.sync.dma_start(o_pairs[64 * hh:64 * (hh + 1), pair, :],
                                      o_collb[0:64, 2 * pair + hh, :])

            def make_norm_chain(q4, o_pairs, ds):
                """Per-q4 normalization + staged O-projection closures.
                part_norm_a/b take a list of pairs so the last q4 can
                normalize pair 0 while pair 1 is still accumulating."""
                qs0 = q4 * 512
                rs = np_.tile([64, 64], F16, tag="rs", name=f"rs_{q4}")
                r0 = np_.tile([2, 2, 512], F16, tag="r0", name=f"r0_{q4}")
                r_bc = np_.tile([128, 2, 512], F16, tag="rbc", name=f"rbc_{q4}")
                o_n = np_.tile([128, 2, 512], F16, tag="onorm", name=f"on_{q4}")

                def part_norm_a(pairs):
                    # reciprocal of the denominators + broadcast DMAs
                    # (HWDGE with a zero-stride partition dim replicates the
                    # single-partition recip row across the 64 head dims).
                    for p in pairs:
                        with nc.allow_low_precision(
                                reason="f16 softmax scale, rel err ~5e-4"):
                            nc.vector.reciprocal(rs[32 * p:32 * p + 16, :],
                                                 ds[32 * p:32 * p + 16, :])
                        nc.sync.dma_start(r0[:, p, :],
                                          rs[32 * p:32 * p + 16, :])
                        for hh in range(2):
                            nc.sync.dma_start(
                                r_bc[64 * hh:64 * (hh + 1), p, :],
                                _part_bcast(r0[hh:hh + 1, p, :], 64))

                def part_norm_b(pairs):
                    for p in pairs:
                        nc.vector.tensor_tensor(o_n[:, p, :], o_pairs[:, p, :],
                                                r_bc[:, p, :], op=OP.mult)

                def part_oproj(chunks, tail=False):
                    for ci, (sc, mcb) in enumerate(chunks):
                        ss = slice(sc * 128, (sc + 1) * 128)
                        ms = slice(mcb * 512, (mcb + 1) * 512)
                        acco = pso.tile([128, 512], F32, tag="acco")
                        for p in range(2):
                            nc.tensor.matmul(acco[:],
                                             o_n[:, p, ss],
                                             wo_t[p][:, ms],
                                             start=(p == 0), stop=(p == 1))
                        outb = op_.tile([128, 512], F16, tag="outb",
                                        name=f"outb_{q4}_{sc}_{mcb}")
                        nc.scalar.copy(outb[:], acco[:])
                        nc.sync.dma_start(
                            partial[qs0 + sc * 128:qs0 + (sc + 1) * 128, ms],
                            outb[:])

                def part_oproj_tail():
                    # the attention stream is over: the sct PSUM banks are
                    # free, so accumulate full [128,1024] rows (2 banks) and
                    # drain with half the instruction count, copies
                    # alternating ACT/DVE
                    for sc in range(4):
                        ss = slice(sc * 128, (sc + 1) * 128)
                        acco2 = psw.tile([128, 2, 512], F32, tag="sct",
                                         name=f"acct_{sc}")
                        for mcb in range(2):
                            ms = slice(mcb * 512, (mcb + 1) * 512)
                            for p in range(2):
                                nc.tensor.matmul(acco2[:, mcb, :],
                                                 o_n[:, p, ss],
                                                 wo_t[p][:, ms],
                                                 start=(p == 0), stop=(p == 1))
                        outb2 = op_.tile([128, 2, 512], F16, tag="outb2",
                                         name=f"outb2_{sc}")
                        if sc % 2 == 1:
                            nc.vector.tensor_copy(outb2[:], acco2[:])
                        else:
                            nc.scalar.copy(outb2[:], acco2[:])
                        nc.sync.dma_start(
                            partial[qs0 + sc * 128:qs0 + (sc + 1) * 128, :],
                            outb2[:])

                return part_norm_a, part_norm_b, part_oproj, part_oproj_tail

            CHUNKS = [(sc, mcb) for sc in range(4) for mcb in range(2)]
            pending = None
            mw = mw0
            for q4 in range(QC):
                # prefetch the NEXT q4's mask during pair 0 (hook 4 for
                # q4>=1; loop-top for q4=0 whose hooks carry the V chunks)
                mw_nxt = None
                if q4 + 1 < QC:
                    mw_nxt = mp.tile([128, KC, 512], F16, tag="mask",
                                     name=f"mw_{q4 + 1}")
                    if q4 == 0:
                        nc.sync.dma_start(mw_nxt[:], mask4[q4 + 1])
                o_collb = np_.tile([65, HPC, 512], F16, tag="ocoll",
                                   name=f"ocoll_{q4}")
                o_pairs = np_.tile([128, 2, 512], F16, tag="opair",
                                   name=f"opairs_{q4}")
                ds = np_.tile([64, 64], F16, tag="ds", name=f"ds_{q4}")

                kb_st = {}

                def _kb(b, half):
                    if half == 0:
                        kb_st[b] = pso.tile([128, 512], F32, tag="acco",
                                            name=f"acck1_{b}")
                    emit_k_block(1, b, kb_st[b], epilogue=(half == 1),
                                 quarters=range(2 * half, 2 * half + 2))

                if pending is not None:
                    pa, pb, po, _pot = pending
                    eqB = {}
                    eq_hooks = ([lambda eqB=eqB, q=q4:
                                 emit_q_half(1, q, eqB, 0, pso, "acco")],
                                [lambda eqB=eqB, q=q4:
                                 emit_q_half(1, q, eqB, 1, pso, "acco")]) \
                        if q4 >= 2 else ([], [])
                    hooks_a = {0: eq_hooks[0],
                               1: eq_hooks[1],
                               3: [lambda pa=pa: pa([0, 1])],
                               4: [lambda mw_nxt=mw_nxt:
                                   nc.sync.dma_start(mw_nxt[:],
                                                     mask4[min(q4 + 1, 3)])
                                   ] if q4 + 1 < QC else [],
                               5: [lambda pb=pb: pb([0, 1])],
                               6: [lambda po=po: po(CHUNKS[0:1])],
                               7: [lambda po=po: po(CHUNKS[1:2])]}
                else:
                    def _vp(sc):
                        emit_v(sc, pso, "acco")

                    k0_st = {}

                    def _k0(b, half):
                        if half == 0:
                            k0_st[b] = pso.tile([128, 512], F32, tag="acco",
                                                name=f"acck0_{b}")
                        emit_k_block(0, b, k0_st[b], epilogue=(half == 1),
                                     quarters=range(2 * half, 2 * half + 2))
                    # NOTE: tiles held across hooks (K/Q accumulators) must
                    # be the LAST "acco" allocation of their first hook and
                    # continue as the FIRST emission of the next hook, so the
                    # 2-buffer rotation never reuses a held bank mid-
                    # accumulation.
                    eq10 = {}
                    hooks_a = {0: [lambda: _vp(0), lambda: _vp(1),
                                   lambda: _k0(1, 0)],
                               1: [lambda: _k0(1, 1),
                                   lambda: _vp(2), lambda: _vp(3)],
                               2: [lambda: _vp(4), lambda: _vp(5),
                                   lambda: _k0(2, 0)],
                               3: [lambda: _k0(2, 1),
                                   lambda: _vp(6), lambda: _vp(7)],
                               4: [lambda: _vp(8), lambda: _vp(9),
                                   lambda: _k0(3, 0)],
                               5: [lambda: _k0(3, 1),
                                   lambda: _vp(10), lambda: _vp(11),
                                   lambda: emit_q_half(1, 0, eq10, 0,
                                                       pso, "acco")],
                               6: [lambda: emit_q_half(1, 0, eq10, 1,
                                                       pso, "acco"),
                                   lambda: _vp(12), lambda: _vp(13),
                                   lambda: _kb(0, 0)],
                               7: [lambda: _kb(0, 1),
                                   lambda: _vp(14), lambda: _vp(15)]}
                accs = emit_pair(q4, 0, mw, hooks_a)
                emit_collect(q4, 0, accs, o_collb, o_pairs, ds)

                hooks_b = {}
                if pending is not None:
                    pa, pb, po, _pot = pending
                    for i in range(6):
                        hooks_b[i] = [lambda po=po, i=i:
                                      po(CHUNKS[2 + i:3 + i])]
                else:
                    hooks_b = {0: [lambda: _kb(1, 0)],
                               1: [lambda: _kb(1, 1)],
                               2: [lambda: _kb(2, 0)],
                               3: [lambda: _kb(2, 1)],
                               4: [lambda: _kb(3, 0)],
                               5: [lambda: _kb(3, 1)]}
                if q4 + 1 < QC:
                    eqA = {}
                    hooks_b.setdefault(6, []).append(
                        lambda eqA=eqA, q=q4 + 1:
                        emit_q_half(0, q, eqA, 0, pso, "acco"))
                    hooks_b.setdefault(7, []).append(
                        lambda eqA=eqA, q=q4 + 1:
                        emit_q_half(0, q, eqA, 1, pso, "acco"))
                    if q4 == 0:
                        eqC = {}
                        hooks_b.setdefault(3, []).append(
                            lambda eqC=eqC: emit_q_half(1, 1, eqC, 0,
                                                        pso, "acco"))
                        hooks_b.setdefault(5, []).append(
                            lambda eqC=eqC: emit_q_half(1, 1, eqC, 1,
                                                        pso, "acco"))
                if q4 == QC - 1:
                    # normalize pair 0 of the last q4 while pair 1 runs
                    pend_last = make_norm_chain(q4, o_pairs, ds)
                    pa_l, pb_l, po_l, _pot_l = pend_last
                    hooks_b.setdefault(3, []).append(
                        lambda pa_l=pa_l: pa_l([0]))
                    hooks_b.setdefault(6, []).append(
                        lambda pb_l=pb_l: pb_l([0]))
                accs = emit_pair(q4, 1, mw, hooks_b)
                emit_collect(q4, 1, accs, o_collb, o_pairs, ds)
                if q4 == QC - 1:
                    pending = pend_last
                    pa_l([1])
                else:
                    pending = make_norm_chain(q4, o_pairs, ds)
                    mw = mw_nxt

            for i in range(48):
                sct_f = psw.tile([128, 2, 512], F32, tag="sct",
                                 name=f"tailwarm_{i}")
                nc.tensor.matmul(sct_f[:, 0, :], warm[0:128, 0:128], warm[:],
                                 start=True, stop=True)
            pa, pb, po, pot = pending
            pb([1])
            pot()

            for ctx in (po_ctx, pa_ctx, ps_ctx, op_ctx, np_ctx, sw_ctx,
                        xp_ctx, mp_ctx):
                ctx.__exit__(None, None, None)

    return nc


_PROGRAM = None


def _get_program():
    global _PROGRAM
    if _PROGRAM is None:
        _PROGRAM = _build_program()
    return _PROGRAM


def _prepare_in_maps(inputs):
    f16 = np.float16
    x = np.asarray(inputs["x"], np.float32)
    mask = np.asarray(inputs["mask"], np.float32)
    Wq = np.asarray(inputs["Wq"], np.float32)
    bq = np.asarray(inputs["bq"], np.float32)
    Wk = np.asarray(inputs["Wk"], np.float32)
    bk = np.asarray(inputs["bk"], np.float32)
    Wv = np.asarray(inputs["Wv"], np.float32)
    Wo = np.asarray(inputs["Wo"], np.float32)

    xT_b = [np.ascontiguousarray(x[b].T).astype(f16) for b in range(B)]
    # mask4[q4, p, c, j] = mask[b,0].T[c*128+p, q4*512+j]
    mask4_b = []
    for b in range(B):
        mt = np.ascontiguousarray(mask[b, 0].T)          # [keys, queries]
        m4 = mt.reshape(KC, 128, QC, 512).transpose(2, 1, 0, 3)
        mask4_b.append(np.ascontiguousarray(m4).astype(f16))

    in_maps = []
    for c in range(NCORES):
        b = c // 4
        h0 = (c % 4) * HPC
        cs = slice(h0 * DK, (h0 + HPC) * DK)
        wq_s = np.ascontiguousarray(Wq[cs, :].T).astype(f16)    # [M, HD]
        wk_s = np.ascontiguousarray(Wk[cs, :].T).astype(f16)
        wv_s = np.ascontiguousarray(Wv[cs, :].T).astype(f16)
        wo_s = np.ascontiguousarray(Wo[:, cs].T).astype(f16)    # [HD, M]
        bq_s = (bq[cs] / 8.0).reshape(2, 128).T.copy().astype(np.float32)
        bk_s = bk[cs].reshape(2, 128).T.copy().astype(np.float32)
        in_maps.append(dict(xT=xT_b[b], mask4=mask4_b[b],
                            wq=wq_s, wk=wk_s, wv=wv_s,
                            wo2=wo_s.reshape(2, 128, M),
                            bq2=bq_s, bk2=bk_s))
    return in_maps


def kernel(x, mask, Wq, bq, Wk, bk, Wv, bv, Wo, bo):
    global LAST_RESULTS
    inputs = dict(x=x, mask=mask, Wq=Wq, bq=bq, Wk=Wk, bk=bk, Wv=Wv, bv=bv,
                  Wo=Wo, bo=bo)
    in_maps = _prepare_in_maps(inputs)

    nc = _get_program()
    res = run_bass_kernel_spmd(nc, in_maps, list(range(NCORES)))
    LAST_RESULTS = res

    out = np.zeros((B, N, M), np.float32)
    for c in range(NCORES):
        out[c // 4] += np.asarray(res.results[c]["partial"], np.float32)
    bv_ = np.asarray(bv, np.float32)
    Wo_ = np.asarray(Wo, np.float32)
    bo_ = np.asarray(bo, np.float32)
    out += (bo_ + bv_ @ Wo_.T)[None, None, :]
    return out


# revision 42
# speedup vs baseline: 1.0996x; 1.0996x over previous
"""Multi-head attention (B=2, N=2048, M=1024, H=16) on 8 trn2 NeuronCores.

Sharding: core c handles batch b = c//4 and heads 4*(c%4) .. 4*(c%4)+4.
Each core computes its 4 heads' attention and a partial output projection;
the host sums the 4 partials per batch and adds the constant bias term
(bo + bv @ Wo.T - exact because softmax rows sum to 1).

Design (fp16 compute, f32 PSUM accumulation):
  - Steady state is jointly DVE/ACT-paced (~39 us per 512-query block):
    mask-multiply (PSUM-f32 source, 1x DVE mode) and softmax exp (ACT) are
    the structural floors; everything else is scheduled into their slack.
  - attnV matmuls are software-pipelined one kc2 block behind their exp
    (pend_av) so each PE instruction's dependencies resolve a block early.
  - Minimal prefix (K block 0 + Q block 0 via token-quartered xt DMAs,
    prioritized DMA order); all remaining K/V/Q projections drip through
    per-kc2 hooks. Held PSUM accumulators obey the tag-rotation rule:
    last allocation of their first hook, first emission of the next.
  - collect drains accs into o_collb [65,4,512] f16 (row 64 = softmax
    denominators); rearrange DMAs (ds, o_pairs) issue at collect time so
    the norm chain (reciprocal -> r0 -> zero-stride broadcast DMAs -> 2x
    f16 multiply) never parks an engine stream on DMA latency.
  - O-projection chunks spread one per hook; next q4's mask prefetched
    mid-pair-0; dummy matmuls on a memset tile hold the PE p-state ramp
    through DMA waits and the final norm lull; the tail O-projection
    batches FD-1024 drains through the freed sct PSUM banks.
"""
import sys
import os

sys.path.insert(0, '/opt/trn_rl_repo')

import numpy as np
import ml_dtypes

import concourse.bass as bass
import concourse.tile as tile
from concourse import mybir
from concourse.vector_clock import ScopedClock
from concourse.bass_utils import run_bass_kernel_spmd

dt = mybir.dt
F32, F16 = dt.float32, dt.float16
AF = mybir.ActivationFunctionType
OP = mybir.AluOpType

B, N, M, H = 2, 2048, 1024, 16
DK = M // H            # 64
HPC = 4                # heads per core
HD = HPC * DK          # 256 head dims per core
NCORES = 8
QC = 4                 # query blocks of 512
KC = 16                # key chunks of 128
MC = 8                 # model-dim chunks of 128
SC = 16                # seq chunks of 128

LAST_RESULTS = None


class TC(tile.TileContext):
    """TileContext patched for a walrus build that only accepts ONE sync-wait
    per instruction: excess waits are peeled onto same-engine NoOps inserted
    immediately before the instruction (engine streams are in-order, so the
    waits still gate the instruction exactly as before)."""
    MAXW = 1

    def _split_waits(self, inst):
        si = inst.sync_info
        if si is None or si.on_wait is None or len(si.on_wait) <= self.MAXW:
            return
        if inst.engine == mybir.EngineType.Unassigned:
            return
        waits = list(si.on_wait)
        for w in waits[:-self.MAXW]:
            nop = mybir.InstNoOp(name=f"nopw-{self.nc.next_id()}", ins=[], outs=[])
            nop.engine = inst.engine
            nop.sync_info = mybir.SyncInfo(on_wait=[w], on_update=[])
            super()._add_instruction(nop)
        si.on_wait = waits[-self.MAXW:]
        inst.sync_info = si

    def _add_instruction(self, inst):
        self._split_waits(inst)
        super()._add_instruction(inst)

    def _drain_and_barrier(self, tick_clock, wait_clock):
        drain_inst = self.nc.sync.drain()
        wait_clock.add_sem_waits(drain_inst.ins,
                                 ScopedClock({None: tick_clock.global_clock}))
        si = drain_inst.ins.sync_info
        if si is not None and si.on_wait is not None and len(si.on_wait) > 1:
            waits = list(si.on_wait)
            si.on_wait = waits[:1]
            drain_inst.ins.sync_info = si
            for w in waits[1:]:
                nop = self.nc.sync.nop(nofuse=True)
                nop.ins.sync_info = mybir.SyncInfo(on_wait=[w], on_update=[])
        self.nc.all_engine_barrier()
        assert self.sems is not None
        popped = self.nc._tile_sem_poison_stack.pop()
        assert popped is self._sem_poison
        self.nc.clear_and_free_semaphores(list(self.sems.allocated().values()))
        self.nc.all_engine_barrier()


def _bcast_mid(ap, n):
    """[P, F] AP -> [P, n, F] AP with a zero-stride middle dim."""
    layout = list(ap.ap)
    assert len(layout) == 2
    new_layout = [layout[0], [0, n], layout[1]]
    return bass.AP(ap.tensor, ap.offset, new_layout)


def _part_bcast(ap, n):
    """[1, F] AP -> [1, n, F] AP replicating the partition's data n times
    via a zero-stride middle dim (DMA src; partition dim keeps step 1)."""
    layout = list(ap.ap)
    assert len(layout) == 2 and layout[0][1] == 1
    new_layout = [layout[0], [0, n], layout[1]]
    return bass.AP(ap.tensor, ap.offset, new_layout)


def _build_program(repeat=1):
    nc = bass.Bass(num_devices=NCORES)

    xT = nc.dram_tensor("xT", [M, N], F16, kind="ExternalInput")
    mask4 = nc.dram_tensor("mask4", [QC, 128, KC, 512], F16, kind="ExternalInput")
    wq = nc.dram_tensor("wq", [M, HD], F16, kind="ExternalInput")   # Wq[slice].T
    wk = nc.dram_tensor("wk", [M, HD], F16, kind="ExternalInput")
    wv = nc.dram_tensor("wv", [M, HD], F16, kind="ExternalInput")
    wo2 = nc.dram_tensor("wo2", [2, 128, M], F16, kind="ExternalInput")  # pair rows
    bq2 = nc.dram_tensor("bq2", [128, 2], F32, kind="ExternalInput")  # bq[slice]/8
    bk2 = nc.dram_tensor("bk2", [128, 2], F32, kind="ExternalInput")
    partial = nc.dram_tensor("partial", [N, M], F16, kind="ExternalOutput")

    with TC(nc) as tc:
      for _rep in range(repeat):
        with tc.tile_pool(name="persist", bufs=1) as pp:
            # ---- persistent tiles ----
            wo_t = [pp.tile([128, M], F16, tag=f"wo{p}", name=f"wo_t{p}")
                    for p in range(2)]
            bq_t = pp.tile([128, 2], F32)
            bk_t = pp.tile([128, 2], F32)

            qT_sb = [pp.tile([128, N], F16, tag=f"qT{pt}", name=f"qT_sb{pt}")
                     for pt in range(2)]
            kT_sb = [pp.tile([128, N], F16, tag=f"kT{pt}", name=f"kT_sb{pt}")
                     for pt in range(2)]
            v_aug = pp.tile([128, SC, HPC, DK + 1], F16)
            warm = pp.tile([128, 512], F16)
            nc.gpsimd.memset(warm[:], 0.0)
            nc.gpsimd.memset(v_aug[:], 1.0)

            # ---- projection inputs ----
            mp_ctx = tc.tile_pool(name="maskp", bufs=2)
            mp = mp_ctx.__enter__()
            xp_ctx = tc.tile_pool(name="projp", bufs=1)
            xp = xp_ctx.__enter__()
            pj_ctx = tc.tile_pool(name="pjps", bufs=1, space="PSUM")
            pj = pj_ctx.__enter__()

            # Startup DMAs in dependency-priority order: the sim (and HW
            # aggregate bandwidth) serializes transfers, so the order below
            # IS the arrival schedule. Split across both HWDGE queues so
            # issue overhead overlaps.
            xt = xp.tile([128, MC, N], F16)
            xt_r = xT.rearrange("(c p) n -> p c n", p=128)
            wk_t = xp.tile([128, MC, HD], F16)
            wq_t = xp.tile([128, MC, HD], F16)
            wv_t = xp.tile([128, MC, HD], F16)
            mw0 = mp.tile([128, KC, 512], F16, tag="mask", name="mw_0")
            nc.sync.dma_start(wk_t[:], wk.rearrange("(c p) h -> p c h", p=128))
            nc.sync.dma_start(bq_t[:], bq2[:])
            nc.sync.dma_start(bk_t[:], bk2[:])
            nc.sync.dma_start(xt[:, :, 0:512], xt_r[:, :, 0:512])
            nc.sync.dma_start(wq_t[:], wq.rearrange("(c p) h -> p c h", p=128))
            nc.sync.dma_start(mw0[:, 0:8, :], mask4[0, :, 0:8, :])
            nc.sync.dma_start(wv_t[:], wv.rearrange("(c p) h -> p c h", p=128))
            for c in range(1, 3):
                nc.sync.dma_start(xt[:, :, 512 * c:512 * (c + 1)],
                                  xt_r[:, :, 512 * c:512 * (c + 1)])
            nc.sync.dma_start(mw0[:, 8:16, :], mask4[0, :, 8:16, :])
            nc.sync.dma_start(xt[:, :, 1536:2048], xt_r[:, :, 1536:2048])
            for p in range(2):
                nc.sync.dma_start(wo_t[p][:], wo2[p])

            # PE p-state prewarm: dummy matmuls on the memset tile keep the
            # ramp clock running through the initial DMA wait.
            pwacc = pj.tile([128, 512], F32, tag="pj3", name="pwacc")
            for _ in range(9):
                nc.tensor.matmul(pwacc[:], warm[:, 0:128], warm[:],
                                 start=True, stop=True)

            def emit_k_block(pt, q4, acck, epilogue=True, quarters=range(4)):
                qs = slice(q4 * 512, (q4 + 1) * 512)
                for qtr in quarters:
                    for mc in (2 * qtr, 2 * qtr + 1):
                        nc.tensor.matmul(acck[:],
                                         wk_t[:, mc, pt * 128:(pt + 1) * 128],
                                         xt[:, mc, qs],
                                         start=(mc == 0), stop=(mc == MC - 1))
                if epilogue:
                    nc.scalar.activation(kT_sb[pt][:, qs], acck[:],
                                         AF.Identity, bias=bk_t[:, pt:pt + 1],
                                         scale=1.0)

            def emit_q_half(pt, q4, st, half, pool, tag):
                qs = slice(q4 * 512, (q4 + 1) * 512)
                if half == 0:
                    st['accq'] = pool.tile([128, 512], F32, tag=tag,
                                           name=f"accq_{pt}_{q4}")
                accq = st['accq']
                for mc in range(half * MC // 2, (half + 1) * MC // 2):
                    nc.tensor.matmul(accq[:],
                                     wq_t[:, mc, pt * 128:(pt + 1) * 128],
                                     xt[:, mc, qs],
                                     start=(mc == 0), stop=(mc == MC - 1))
                if half == 1:
                    nc.scalar.activation(qT_sb[pt][:, qs], accq[:],
                                         AF.Identity, bias=bq_t[:, pt:pt + 1],
                                         scale=0.125)

            def emit_q(pt, q4, pool, tag):
                st = {}
                emit_q_half(pt, q4, st, 0, pool, tag)
                emit_q_half(pt, q4, st, 1, pool, tag)

            def emit_v(sc, pool, tag):
                accv = pool.tile([128, 512], F32, tag=tag)
                for mc in range(MC):
                    nc.tensor.matmul(accv[:, 0:HD],
                                     xt[:, mc, sc * 128:(sc + 1) * 128],
                                     wv_t[:, mc, :],
                                     start=(mc == 0), stop=(mc == MC - 1))
                nc.vector.tensor_copy(v_aug[:, sc, :, 0:DK], accv[:, 0:HD])

            # Minimal prefix: K-pt0 (xt arrives in quarters, so interleave the
            # quarter chunks across the four key blocks), Q0-pt0, then the
            # first pair-1 requirements (K-pt1 block 0, Q0-pt1) and V0/V1.
            acck00 = pj.tile([128, 512], F32, tag="pj0", name="acck00")
            emit_k_block(0, 0, acck00)
            emit_q(0, 0, pj, "pj1")
            pj_ctx.__exit__(None, None, None)

            # ---- attention ----
            sw_ctx = tc.tile_pool(name="sbwork", bufs=3)
            sw = sw_ctx.__enter__()
            np_ctx = tc.tile_pool(name="normp", bufs=2)
            np_ = np_ctx.__enter__()
            op_ctx = tc.tile_pool(name="outp", bufs=3)
            op_ = op_ctx.__enter__()
            ps_ctx = tc.tile_pool(name="pssct", bufs=2, space="PSUM")
            psw = ps_ctx.__enter__()
            pa_ctx = tc.tile_pool(name="psacc", bufs=1, space="PSUM")
            psa = pa_ctx.__enter__()
            po_ctx = tc.tile_pool(name="psout", bufs=2, space="PSUM")
            pso = po_ctx.__enter__()

            def emit_pair(q4, pair, mw, hooks=None, defer_av=False):
                """scores -> mask-mul -> exp -> attnV accumulate for 2 heads.
                hooks[kc2] is a list of closures emitted before that kc2
                block, spreading other work into the steady-state slack.
                defer_av shifts each kc2's attnV matmuls one block later so
                hook-emitted v_aug chunks may land inside this pair without
                deadlocking the in-order PE stream."""
                qs = slice(q4 * 512, (q4 + 1) * 512)
                accs = psa.tile([65, 2, 512], F32, tag="accs",
                                name=f"accs_{q4}_{pair}")

                def emit_av(kc2, pe):
                    for kci in range(2):
                        kc = kc2 * 2 + kci
                        for hh in range(2):
                            nc.tensor.matmul(accs[:, hh, :],
                                             v_aug[:, kc, 2 * pair + hh, :],
                                             pe[:, kci, hh, :],
                                             start=(kc == 0), stop=(kc == KC - 1))

                pend_av = None
                for kc2 in range(KC // 2):
                    if hooks and kc2 in hooks:
                        for h in hooks[kc2]:
                            h()
                    pm = sw.tile([128, 2, 2, 512], F16, tag="pm",
                                 name=f"pm_{q4}_{pair}_{kc2}")
                    for kci in range(2):
                        kc = kc2 * 2 + kci
                        ks = slice(kc * 128, (kc + 1) * 128)
                        sct = psw.tile([128, 2, 512], F32, tag="sct",
                                       name=f"sct_{q4}_{pair}_{kc}")
                        nc.tensor.matmul(sct[:, 0, :],
                                         kT_sb[pair][0:64, ks],
                                         qT_sb[pair][0:64, qs],
                                         start=True, stop=True)
                        nc.tensor.matmul(sct[:, 1, :],
                                         kT_sb[pair][64:128, ks],
                                         qT_sb[pair][64:128, qs],
                                         start=True, stop=True)
                        nc.vector.tensor_tensor(pm[:, kci], sct[:],
                                                _bcast_mid(mw[:, kc, :], 2),
                                                op=OP.mult)
                    if pend_av is not None:
                        emit_av(*pend_av)
                        pend_av = None
                    pe = sw.tile([128, 2, 2, 512], F16, tag="pe",
                                 name=f"pe_{q4}_{pair}_{kc2}")
                    nc.scalar.activation(pe[:], pm[:], AF.Exp)
                    pend_av = (kc2, pe)
                if pend_av is not None:
                    emit_av(*pend_av)
                return accs

            def emit_collect(q4, pair, accs, o_collb, o_pairs, ds):
                """Drain the pair's accumulators (frees accs) and issue the
                rearrange DMAs now so the norm chain finds them resolved."""
                for hh in range(2):
                    nc.scalar.copy(o_collb[:, 2 * pair + hh, :],
                                   accs[:, hh, :])
                nc.sync.dma_start(ds[32 * pair:32 * pair + 16, :],
                                  o_collb[64:65, 2 * pair:2 * pair + 2, :])
                for hh in range(2):
                    nc.sync.dma_start(o_pairs[64 * hh:64 * (hh + 1), pair, :],
                                      o_collb[0:64, 2 * pair + hh, :])

            def make_norm_chain(q4, o_pairs, ds):
                """Per-q4 normalization + staged O-projection closures.
                part_norm_a/b take a list of pairs so the last q4 can
                normalize pair 0 while pair 1 is still accumulating."""
                qs0 = q4 * 512
                rs = np_.tile([64, 64], F16, tag="rs", name=f"rs_{q4}")
                r0 = np_.tile([2, 2, 512], F16, tag="r0", name=f"r0_{q4}")
                r_bc = np_.tile([128, 2, 512], F16, tag="rbc", name=f"rbc_{q4}")
                o_n = np_.tile([128, 2, 512], F16, tag="onorm", name=f"on_{q4}")

                def part_norm_a(pairs):
                    # reciprocal of the denominators + broadcast DMAs
                    # (HWDGE with a zero-stride partition dim replicates the
                    # single-partition recip row across the 64 head dims).
                    for p in pairs:
                        with nc.allow_low_precision(
                                reason="f16 softmax scale, rel err ~5e-4"):
                            nc.vector.reciprocal(rs[32 * p:32 * p + 16, :],
                                                 ds[32 * p:32 * p + 16, :])
                        nc.sync.dma_start(r0[:, p, :],
                                          rs[32 * p:32 * p + 16, :])
                        for hh in range(2):
                            nc.sync.dma_start(
                                r_bc[64 * hh:64 * (hh + 1), p, :],
                                _part_bcast(r0[hh:hh + 1, p, :], 64))

                def part_norm_b(pairs):
                    for p in pairs:
                        nc.vector.tensor_tensor(o_n[:, p, :], o_pairs[:, p, :],
                                                r_bc[:, p, :], op=OP.mult)

                def part_oproj(chunks, tail=False):
                    for ci, (sc, mcb) in enumerate(chunks):
                        ss = slice(sc * 128, (sc + 1) * 128)
                        ms = slice(mcb * 512, (mcb + 1) * 512)
                        acco = pso.tile([128, 512], F32, tag="acco")
                        for p in range(2):
                            nc.tensor.matmul(acco[:],
                                             o_n[:, p, ss],
                                             wo_t[p][:, ms],
                                             start=(p == 0), stop=(p == 1))
                        outb = op_.tile([128, 512], F16, tag="outb",
                                        name=f"outb_{q4}_{sc}_{mcb}")
                        nc.scalar.copy(outb[:], acco[:])
                        nc.sync.dma_start(
                            partial[qs0 + sc * 128:qs0 + (sc + 1) * 128, ms],
                            outb[:])

                def part_oproj_tail():
                    # the attention stream is over: the sct PSUM banks are
                    # free, so accumulate full [128,1024] rows (2 banks) and
                    # drain with half the instruction count, copies
                    # alternating ACT/DVE
                    for sc in range(4):
                        ss = slice(sc * 128, (sc + 1) * 128)
                        acco2 = psw.tile([128, 2, 512], F32, tag="sct",
                                         name=f"acct_{sc}")
                        for mcb in range(2):
                            ms = slice(mcb * 512, (mcb + 1) * 512)
                            for p in range(2):
                                nc.tensor.matmul(acco2[:, mcb, :],
                                                 o_n[:, p, ss],
                                                 wo_t[p][:, ms],
                                                 start=(p == 0), stop=(p == 1))
                        outb2 = op_.tile([128, 2, 512], F16, tag="outb2",
                                         name=f"outb2_{sc}")
                        if sc % 2 == 1:
                            nc.vector.tensor_copy(outb2[:], acco2[:])
                        else:
                            nc.scalar.copy(outb2[:], acco2[:])
                        nc.sync.dma_start(
                            partial[qs0 + sc * 128:qs0 + (sc + 1) * 128, :],
                            outb2[:])

                return part_norm_a, part_norm_b, part_oproj, part_oproj_tail

            CHUNKS = [(sc, mcb) for sc in range(4) for mcb in range(2)]
            pending = None
            mw = mw0
            for q4 in range(QC):
                # prefetch the NEXT q4's mask during pair 0 (hook 4 for
                # q4>=1; loop-top for q4=0 whose hooks carry the V chunks)
                mw_nxt = None
                if q4 + 1 < QC:
                    mw_nxt = mp.tile([128, KC, 512], F16, tag="mask",
                                     name=f"mw_{q4 + 1}")
                    if q4 == 0:
                        nc.sync.dma_start(mw_nxt[:], mask4[q4 + 1])
                o_collb = np_.tile([65, HPC, 512], F16, tag="ocoll",
                                   name=f"ocoll_{q4}")
                o_pairs = np_.tile([128, 2, 512], F16, tag="opair",
                                   name=f"opairs_{q4}")
                ds = np_.tile([64, 64], F16, tag="ds", name=f"ds_{q4}")

                kb_st = {}

                def _kb(b, half):
                    if half == 0:
                        kb_st[b] = pso.tile([128, 512], F32, tag="acco",
                                            name=f"acck1_{b}")
                    emit_k_block(1, b, kb_st[b], epilogue=(half == 1),
                                 quarters=range(2 * half, 2 * half + 2))

                if pending is not None:
                    pa, pb, po, _pot = pending
                    eqB = {}
                    eq_hooks = ([lambda eqB=eqB, q=q4:
                                 emit_q_half(1, q, eqB, 0, pso, "acco")],
                                [lambda eqB=eqB, q=q4:
                                 emit_q_half(1, q, eqB, 1, pso, "acco")]) \
                        if q4 >= 2 else ([], [])
                    hooks_a = {0: eq_hooks[0],
                               1: eq_hooks[1],
                               3: [lambda pa=pa: pa([0, 1])],
                               4: [lambda mw_nxt=mw_nxt:
                                   nc.sync.dma_start(mw_nxt[:],
                                                     mask4[min(q4 + 1, 3)])
                                   ] if q4 + 1 < QC else [],
                               5: [lambda pb=pb: pb([0, 1])],
                               6: [lambda po=po: po(CHUNKS[0:1])],
                               7: [lambda po=po: po(CHUNKS[1:2])]}
                else:
                    def _vp(sc):
                        emit_v(sc, pso, "acco")

                    k0_st = {}

                    def _k0(b, half):
                        if half == 0:
                            k0_st[b] = pso.tile([128, 512], F32, tag="acco",
                                                name=f"acck0_{b}")
                        emit_k_block(0, b, k0_st[b], epilogue=(half == 1),
                                     quarters=range(2 * half, 2 * half + 2))
                    # NOTE: tiles held across hooks (K/Q accumulators) must
                    # be the LAST "acco" allocation of their first hook and
                    # continue as the FIRST emission of the next hook, so the
                    # 2-buffer rotation never reuses a held bank mid-
                    # accumulation.
                    eq10 = {}
                    hooks_a = {0: [lambda: _vp(0), lambda: _vp(1),
                                   lambda: _k0(1, 0)],
                               1: [lambda: _k0(1, 1),
                                   lambda: _vp(2), lambda: _vp(3)],
                               2: [lambda: _vp(4), lambda: _vp(5),
                                   lambda: _k0(2, 0)],
                               3: [lambda: _k0(2, 1),
                                   lambda: _vp(6), lambda: _vp(7)],
                               4: [lambda: _vp(8), lambda: _vp(9),
                                   lambda: _k0(3, 0)],
                               5: [lambda: _k0(3, 1),
                                   lambda: _vp(10), lambda: _vp(11),
                                   lambda: emit_q_half(1, 0, eq10, 0,
                                                       pso, "acco")],
                               6: [lambda: emit_q_half(1, 0, eq10, 1,
                                                       pso, "acco"),
                                   lambda: _vp(12), lambda: _vp(13),
                                   lambda: _kb(0, 0)],
                               7: [lambda: _kb(0, 1),
                                   lambda: _vp(14), lambda: _vp(15)]}
                accs = emit_pair(q4, 0, mw, hooks_a)
                emit_collect(q4, 0, accs, o_collb, o_pairs, ds)

                hooks_b = {}
                if pending is not None:
                    pa, pb, po, _pot = pending
                    for i in range(6):
                        hooks_b[i] = [lambda po=po, i=i:
                                      po(CHUNKS[2 + i:3 + i])]
                else:
                    hooks_b = {0: [lambda: _kb(1, 0)],
                               1: [lambda: _kb(1, 1)],
                               2: [lambda: _kb(2, 0)],
                               3: [lambda: _kb(2, 1)],
                               4: [lambda: _kb(3, 0)],
                               5: [lambda: _kb(3, 1)]}
                if q4 + 1 < QC:
                    eqA = {}
                    hooks_b.setdefault(6, []).append(
                        lambda eqA=eqA, q=q4 + 1:
                        emit_q_half(0, q, eqA, 0, pso, "acco"))
                    hooks_b.setdefault(7, []).append(
                        lambda eqA=eqA, q=q4 + 1:
                        emit_q_half(0, q, eqA, 1, pso, "acco"))
                    if q4 == 0:
                        eqC = {}
                        hooks_b.setdefault(3, []).append(
                            lambda eqC=eqC: emit_q_half(1, 1, eqC, 0,
                                                        pso, "acco"))
                        hooks_b.setdefault(5, []).append(
                            lambda eqC=eqC: emit_q_half(1, 1, eqC, 1,
                                                        pso, "acco"))
                if q4 == QC - 1:
                    # normalize pair 0 of the last q4 while pair 1 runs
                    pend_last = make_norm_chain(q4, o_pairs, ds)
                    pa_l, pb_l, po_l, _pot_l = pend_last
                    hooks_b.setdefault(3, []).append(
                        lambda pa_l=pa_l: pa_l([0]))
                    hooks_b.setdefault(6, []).append(
                        lambda pb_l=pb_l: pb_l([0]))
                accs = emit_pair(q4, 1, mw, hooks_b)
                emit_collect(q4, 1, accs, o_collb, o_pairs, ds)
                if q4 == QC - 1:
                    pending = pend_last
                    pa_l([1])
                else:
                    pending = make_norm_chain(q4, o_pairs, ds)
                    mw = mw_nxt

            for i in range(48):
                sct_f = psw.tile([128, 2, 512], F32, tag="sct",
                                 name=f"tailwarm_{i}")
                nc.tensor.matmul(sct_f[:, 0, :], warm[0:128, 0:128], warm[:],
                                 start=True, stop=True)
            pa, pb, po, pot = pending
            pb([1])
            pot()

            for ctx in (po_ctx, pa_ctx, ps_ctx, op_ctx, np_ctx, sw_ctx,
                        xp_ctx, mp_ctx):
                ctx.__exit__(None, None, None)

    return nc


_PROGRAM = None


def _get_program():
    global _PROGRAM
    if _PROGRAM is None:
        _PROGRAM = _build_program()
    return _PROGRAM


def _prepare_in_maps(inputs):
    f16 = np.float16
    x = np.asarray(inputs["x"], np.float32)
    mask = np.asarray(inputs["mask"], np.float32)
    Wq = np.asarray(inputs["Wq"], np.float32)
    bq = np.asarray(inputs["bq"], np.float32)
    Wk = np.asarray(inputs["Wk"], np.float32)
    bk = np.asarray(inputs["bk"], np.float32)
    Wv = np.asarray(inputs["Wv"], np.float32)
    Wo = np.asarray(inputs["Wo"], np.float32)

    xT_b = [np.ascontiguousarray(x[b].T).astype(f16) for b in range(B)]
    # mask4[q4, p, c, j] = mask[b,0].T[c*128+p, q4*512+j]
    mask4_b = []
    for b in range(B):
        mt = np.ascontiguousarray(mask[b, 0].T)          # [keys, queries]
        m4 = mt.reshape(KC, 128, QC, 512).transpose(2, 1, 0, 3)
        mask4_b.append(np.ascontiguousarray(m4).astype(f16))

    in_maps = []
    for c in range(NCORES):
        b = c // 4
        h0 = (c % 4) * HPC
        cs = slice(h0 * DK, (h0 + HPC) * DK)
        wq_s = np.ascontiguousarray(Wq[cs, :].T).astype(f16)    # [M, HD]
        wk_s = np.ascontiguousarray(Wk[cs, :].T).astype(f16)
        wv_s = np.ascontiguousarray(Wv[cs, :].T).astype(f16)
        wo_s = np.ascontiguousarray(Wo[:, cs].T).astype(f16)    # [HD, M]
        bq_s = (bq[cs] / 8.0).reshape(2, 128).T.copy().astype(np.float32)
        bk_s = bk[cs].reshape(2, 128).T.copy().astype(np.float32)
        in_maps.append(dict(xT=xT_b[b], mask4=mask4_b[b],
                            wq=wq_s, wk=wk_s, wv=wv_s,
                            wo2=wo_s.reshape(2, 128, M),
                            bq2=bq_s, bk2=bk_s))
    return in_maps


def kernel(x, mask, Wq, bq, Wk, bk, Wv, bv, Wo, bo):
    global LAST_RESULTS
    inputs = dict(x=x, mask=mask, Wq=Wq, bq=bq, Wk=Wk, bk=bk, Wv=Wv, bv=bv,
                  Wo=Wo, bo=bo)
    in_maps = _prepare_in_maps(inputs)

    nc = _get_program()
    res = run_bass_kernel_spmd(nc, in_maps, list(range(NCORES)))
    LAST_RESULTS = res

    out = np.zeros((B, N, M), np.float32)
    for c in range(NCORES):
        out[c // 4] += np.asarray(res.results[c]["partial"], np.float32)
    bv_ = np.asarray(bv, np.float32)
    Wo_ = np.asarray(Wo, np.float32)
    bo_ = np.asarray(bo, np.float32)
    out += (bo_ + bv_ @ Wo_.T)[None, None, :]
    return out


# revision 47
# speedup vs baseline: 1.1738x; 1.0675x over previous
"""Multi-head attention (B=2, N=2048, M=1024, H=16) on 8 trn2 NeuronCores.

Sharding: core c handles batch b = c//4 and heads 4*(c%4) .. 4*(c%4)+4.
Each core computes its 4 heads' attention and a partial output projection;
the host sums the 4 partials per batch and adds the constant bias term
(bo + bv @ Wo.T - exact because softmax rows sum to 1).

Design (fp16 compute, f32 PSUM accumulation):
  - Steady state is jointly DVE/ACT-paced (~39 us per 512-query block):
    mask-multiply (PSUM-f32 source, 1x DVE mode) and softmax exp (ACT) are
    the structural floors; everything else is scheduled into their slack.
  - attnV matmuls are software-pipelined one kc2 block behind their exp
    (pend_av) so each PE instruction's dependencies resolve a block early.
  - Minimal prefix (K block 0 + Q block 0 via token-quartered xt DMAs,
    prioritized DMA order); all remaining K/V/Q projections drip through
    per-kc2 hooks. Held PSUM accumulators obey the tag-rotation rule:
    last allocation of their first hook, first emission of the next.
  - collect drains accs into o_collb [65,4,512] f16 (row 64 = softmax
    denominators); rearrange DMAs (ds, o_pairs) issue at collect time so
    the norm chain (reciprocal -> r0 -> zero-stride broadcast DMAs -> 2x
    f16 multiply) never parks an engine stream on DMA latency.
  - O-projection chunks spread one per hook; next q4's mask prefetched
    mid-pair-0; dummy matmuls on a memset tile hold the PE p-state ramp
    through DMA waits and the final norm lull; the tail O-projection
    batches FD-1024 drains through the freed sct PSUM banks.
"""
import sys
import os

sys.path.insert(0, '/opt/trn_rl_repo')

import numpy as np
import ml_dtypes

import concourse.bass as bass
import concourse.tile as tile
from concourse import mybir
from concourse.vector_clock import ScopedClock
from concourse.bass_utils import run_bass_kernel_spmd

dt = mybir.dt
F32, F16 = dt.float32, dt.float16
AF = mybir.ActivationFunctionType
OP = mybir.AluOpType

B, N, M, H = 2, 2048, 1024, 16
DK = M // H            # 64
HPC = 4                # heads per core
HD = HPC * DK          # 256 head dims per core
NCORES = 8
QC = 4                 # query blocks of 512
KC = 16                # key chunks of 128
MC = 8                 # model-dim chunks of 128
SC = 16                # seq chunks of 128

LAST_RESULTS = None


class TC(tile.TileContext):
    """TileContext patched for a walrus build that only accepts ONE sync-wait
    per instruction: excess waits are peeled onto same-engine NoOps inserted
    immediately before the instruction (engine streams are in-order, so the
    waits still gate the instruction exactly as before)."""
    MAXW = 1

    def _split_waits(self, inst):
        si = inst.sync_info
        if si is None or si.on_wait is None or len(si.on_wait) <= self.MAXW:
            return
        if inst.engine == mybir.EngineType.Unassigned:
            return
        waits = list(si.on_wait)
        for w in waits[:-self.MAXW]:
            nop = mybir.InstNoOp(name=f"nopw-{self.nc.next_id()}", ins=[], outs=[])
            nop.engine = inst.engine
            nop.sync_info = mybir.SyncInfo(on_wait=[w], on_update=[])
            super()._add_instruction(nop)
        si.on_wait = waits[-self.MAXW:]
        inst.sync_info = si

    def _add_instruction(self, inst):
        self._split_waits(inst)
        super()._add_instruction(inst)

    def _drain_and_barrier(self, tick_clock, wait_clock):
        drain_inst = self.nc.sync.drain()
        wait_clock.add_sem_waits(drain_inst.ins,
                                 ScopedClock({None: tick_clock.global_clock}))
        si = drain_inst.ins.sync_info
        if si is not None and si.on_wait is not None and len(si.on_wait) > 1:
            waits = list(si.on_wait)
            si.on_wait = waits[:1]
            drain_inst.ins.sync_info = si
            for w in waits[1:]:
                nop = self.nc.sync.nop(nofuse=True)
                nop.ins.sync_info = mybir.SyncInfo(on_wait=[w], on_update=[])
        self.nc.all_engine_barrier()
        assert self.sems is not None
        popped = self.nc._tile_sem_poison_stack.pop()
        assert popped is self._sem_poison
        self.nc.clear_and_free_semaphores(list(self.sems.allocated().values()))
        self.nc.all_engine_barrier()


def _bcast_mid(ap, n):
    """[P, F] AP -> [P, n, F] AP with a zero-stride middle dim."""
    layout = list(ap.ap)
    assert len(layout) == 2
    new_layout = [layout[0], [0, n], layout[1]]
    return bass.AP(ap.tensor, ap.offset, new_layout)


def _part_bcast(ap, n):
    """[1, F] AP -> [1, n, F] AP replicating the partition's data n times
    via a zero-stride middle dim (DMA src; partition dim keeps step 1)."""
    layout = list(ap.ap)
    assert len(layout) == 2 and layout[0][1] == 1
    new_layout = [layout[0], [0, n], layout[1]]
    return bass.AP(ap.tensor, ap.offset, new_layout)


def _build_program(repeat=1):
    nc = bass.Bass(num_devices=NCORES)

    xT = nc.dram_tensor("xT", [M, N], F16, kind="ExternalInput")
    mask4 = nc.dram_tensor("mask4", [QC, 128, KC, 512], F16, kind="ExternalInput")
    wq = nc.dram_tensor("wq", [M, HD], F16, kind="ExternalInput")   # Wq[slice].T
    wk = nc.dram_tensor("wk", [M, HD], F16, kind="ExternalInput")
    wv = nc.dram_tensor("wv", [M, HD], F16, kind="ExternalInput")
    wo2 = nc.dram_tensor("wo2", [2, 128, M], F16, kind="ExternalInput")  # pair rows
    bq2 = nc.dram_tensor("bq2", [128, 2], F32, kind="ExternalInput")  # bq[slice]/8
    bk2 = nc.dram_tensor("bk2", [128, 2], F32, kind="ExternalInput")
    partial = nc.dram_tensor("partial", [N, M], F16, kind="ExternalOutput")

    with TC(nc) as tc:
      for _rep in range(repeat):
        with tc.tile_pool(name="persist", bufs=1) as pp:
            # ---- persistent tiles ----
            wo_t = [pp.tile([128, M], F16, tag=f"wo{p}", name=f"wo_t{p}")
                    for p in range(2)]
            bq_t = pp.tile([128, 2], F32)
            bk_t = pp.tile([128, 2], F32)

            qT_sb = [pp.tile([128, N], F16, tag=f"qT{pt}", name=f"qT_sb{pt}")
                     for pt in range(2)]
            kT_sb = [pp.tile([128, N], F16, tag=f"kT{pt}", name=f"kT_sb{pt}")
                     for pt in range(2)]
            v_aug = pp.tile([128, SC, HPC, DK + 1], F16)
            warm = pp.tile([128, 512], F16)
            nc.gpsimd.memset(warm[:], 0.0)
            nc.gpsimd.memset(v_aug[:], 1.0)

            # ---- projection inputs ----
            mp_ctx = tc.tile_pool(name="maskp", bufs=2)
            mp = mp_ctx.__enter__()
            xp_ctx = tc.tile_pool(name="projp", bufs=1)
            xp = xp_ctx.__enter__()
            pj_ctx = tc.tile_pool(name="pjps", bufs=1, space="PSUM")
            pj = pj_ctx.__enter__()

            # Startup DMAs in dependency-priority order: the sim (and HW
            # aggregate bandwidth) serializes transfers, so the order below
            # IS the arrival schedule. Split across both HWDGE queues so
            # issue overhead overlaps.
            xt = xp.tile([128, MC, N], F16)
            xt_r = xT.rearrange("(c p) n -> p c n", p=128)
            wk_t = xp.tile([128, MC, HD], F16)
            wq_t = xp.tile([128, MC, HD], F16)
            wv_t = xp.tile([128, MC, HD], F16)
            mw0 = mp.tile([128, KC, 512], F16, tag="mask", name="mw_0")
            nc.sync.dma_start(wk_t[:], wk.rearrange("(c p) h -> p c h", p=128))
            nc.sync.dma_start(bq_t[:], bq2[:])
            nc.sync.dma_start(bk_t[:], bk2[:])
            nc.sync.dma_start(xt[:, :, 0:512], xt_r[:, :, 0:512])
            nc.sync.dma_start(wq_t[:], wq.rearrange("(c p) h -> p c h", p=128))
            nc.sync.dma_start(mw0[:, 0:8, :], mask4[0, :, 0:8, :])
            nc.sync.dma_start(wv_t[:], wv.rearrange("(c p) h -> p c h", p=128))
            for c in range(1, 3):
                nc.sync.dma_start(xt[:, :, 512 * c:512 * (c + 1)],
                                  xt_r[:, :, 512 * c:512 * (c + 1)])
            nc.sync.dma_start(mw0[:, 8:16, :], mask4[0, :, 8:16, :])
            nc.sync.dma_start(xt[:, :, 1536:2048], xt_r[:, :, 1536:2048])
            for p in range(2):
                nc.sync.dma_start(wo_t[p][:], wo2[p])

            # PE p-state prewarm: dummy matmuls on the memset tile keep the
            # ramp clock running through the initial DMA wait.
            pwacc = pj.tile([128, 512], F32, tag="pj3", name="pwacc")
            for _ in range(9):
                nc.tensor.matmul(pwacc[:], warm[:, 0:128], warm[:],
                                 start=True, stop=True)

            def emit_k_block(pt, q4, acck, epilogue=True, quarters=range(4)):
                qs = slice(q4 * 512, (q4 + 1) * 512)
                for qtr in quarters:
                    for mc in (2 * qtr, 2 * qtr + 1):
                        nc.tensor.matmul(acck[:],
                                         wk_t[:, mc, pt * 128:(pt + 1) * 128],
                                         xt[:, mc, qs],
                                         start=(mc == 0), stop=(mc == MC - 1))
                if epilogue:
                    nc.scalar.activation(kT_sb[pt][:, qs], acck[:],
                                         AF.Identity, bias=bk_t[:, pt:pt + 1],
                                         scale=1.0)

            def emit_q_half(pt, q4, st, half, pool, tag):
                qs = slice(q4 * 512, (q4 + 1) * 512)
                if half == 0:
                    st['accq'] = pool.tile([128, 512], F32, tag=tag,
                                           name=f"accq_{pt}_{q4}")
                accq = st['accq']
                for mc in range(half * MC // 2, (half + 1) * MC // 2):
                    nc.tensor.matmul(accq[:],
                                     wq_t[:, mc, pt * 128:(pt + 1) * 128],
                                     xt[:, mc, qs],
                                     start=(mc == 0), stop=(mc == MC - 1))
                if half == 1:
                    nc.scalar.activation(qT_sb[pt][:, qs], accq[:],
                                         AF.Identity, bias=bq_t[:, pt:pt + 1],
                                         scale=0.125)

            def emit_q(pt, q4, pool, tag):
                st = {}
                emit_q_half(pt, q4, st, 0, pool, tag)
                emit_q_half(pt, q4, st, 1, pool, tag)

            def emit_v(sc, pool, tag):
                accv = pool.tile([128, 512], F32, tag=tag)
                for mc in range(MC):
                    nc.tensor.matmul(accv[:, 0:HD],
                                     xt[:, mc, sc * 128:(sc + 1) * 128],
                                     wv_t[:, mc, :],
                                     start=(mc == 0), stop=(mc == MC - 1))
                nc.vector.tensor_copy(v_aug[:, sc, :, 0:DK], accv[:, 0:HD])

            # Minimal prefix: K-pt0 (xt arrives in quarters, so interleave the
            # quarter chunks across the four key blocks), Q0-pt0, then the
            # first pair-1 requirements (K-pt1 block 0, Q0-pt1) and V0/V1.
            acck00 = pj.tile([128, 512], F32, tag="pj0", name="acck00")
            emit_k_block(0, 0, acck00)
            emit_q(0, 0, pj, "pj1")
            pj_ctx.__exit__(None, None, None)

            # ---- attention ----
            sw_ctx = tc.tile_pool(name="sbwork", bufs=4)
            sw = sw_ctx.__enter__()
            np_ctx = tc.tile_pool(name="normp", bufs=2)
            np_ = np_ctx.__enter__()
            op_ctx = tc.tile_pool(name="outp", bufs=3)
            op_ = op_ctx.__enter__()
            ps_ctx = tc.tile_pool(name="pssct", bufs=2, space="PSUM")
            psw = ps_ctx.__enter__()
            pa_ctx = tc.tile_pool(name="psacc", bufs=1, space="PSUM")
            psa = pa_ctx.__enter__()
            po_ctx = tc.tile_pool(name="psout", bufs=2, space="PSUM")
            pso = po_ctx.__enter__()

            def emit_pair(q4, pair, mw, hooks=None, defer_av=False):
                """scores -> mask-mul -> exp -> attnV accumulate for 2 heads.
                hooks[kc2] is a list of closures emitted before that kc2
                block, spreading other work into the steady-state slack.
                defer_av shifts each kc2's attnV matmuls one block later so
                hook-emitted v_aug chunks may land inside this pair without
                deadlocking the in-order PE stream."""
                qs = slice(q4 * 512, (q4 + 1) * 512)
                accs = psa.tile([65, 2, 512], F32, tag="accs",
                                name=f"accs_{q4}_{pair}")

                def emit_av(kc2, pe):
                    for kci in range(2):
                        kc = kc2 * 2 + kci
                        for hh in range(2):
                            nc.tensor.matmul(accs[:, hh, :],
                                             v_aug[:, kc, 2 * pair + hh, :],
                                             pe[:, kci, hh, :],
                                             start=(kc == 0), stop=(kc == KC - 1))

                pend_av = None
                for kc2 in range(KC // 2):
                    if hooks and kc2 in hooks:
                        for h in hooks[kc2]:
                            h()
                    pm = sw.tile([128, 2, 2, 512], F16, tag="pm",
                                 name=f"pm_{q4}_{pair}_{kc2}")
                    for kci in range(2):
                        kc = kc2 * 2 + kci
                        ks = slice(kc * 128, (kc + 1) * 128)
                        sct = psw.tile([128, 2, 512], F32, tag="sct",
                                       name=f"sct_{q4}_{pair}_{kc}")
                        nc.tensor.matmul(sct[:, 0, :],
                                         kT_sb[pair][0:64, ks],
                                         qT_sb[pair][0:64, qs],
                                         start=True, stop=True)
                        nc.tensor.matmul(sct[:, 1, :],
                                         kT_sb[pair][64:128, ks],
                                         qT_sb[pair][64:128, qs],
                                         start=True, stop=True)
                        nc.vector.tensor_tensor(pm[:, kci], sct[:],
                                                _bcast_mid(mw[:, kc, :], 2),
                                                op=OP.mult)
                    if kc2 == 0 and carry is not None:
                        carry()
                    if pend_av is not None:
                        emit_av(*pend_av)
                        pend_av = None
                    pe = sw.tile([128, 2, 2, 512], F16, tag="pe",
                                 name=f"pe_{q4}_{pair}_{kc2}")
                    if kc2 == KC // 2 - 1:
                        # split the last exp per kc so the final attnV (and
                        # the collect behind it) can start a block earlier
                        for kci in range(2):
                            nc.scalar.activation(pe[:, kci], pm[:, kci],
                                                 AF.Exp)
                    else:
                        nc.scalar.activation(pe[:], pm[:], AF.Exp)
                    pend_av = (kc2, pe)
                return accs, (emit_av, pend_av)

            def emit_collect(q4, pair, accs, o_collb, o_pairs, ds):
                """Drain the pair's accumulators (frees accs) and issue the
                rearrange DMAs now so the norm chain finds them resolved."""
                for hh in range(2):
                    nc.scalar.copy(o_collb[:, 2 * pair + hh, :],
                                   accs[:, hh, :])
                nc.sync.dma_start(ds[32 * pair:32 * pair + 16, :],
                                  o_collb[64:65, 2 * pair:2 * pair + 2, :])
                for hh in range(2):
                    nc.sync.dma_start(o_pairs[64 * hh:64 * (hh + 1), pair, :],
                                      o_collb[0:64, 2 * pair + hh, :])

            def make_norm_chain(q4, o_pairs, ds):
                """Per-q4 normalization + staged O-projection closures.
                part_norm_a/b take a list of pairs so the last q4 can
                normalize pair 0 while pair 1 is still accumulating."""
                qs0 = q4 * 512
                rs = np_.tile([64, 64], F16, tag="rs", name=f"rs_{q4}")
                r0 = np_.tile([2, 2, 512], F16, tag="r0", name=f"r0_{q4}")
                r_bc = np_.tile([128, 2, 512], F16, tag="rbc", name=f"rbc_{q4}")
                o_n = np_.tile([128, 2, 512], F16, tag="onorm", name=f"on_{q4}")

                def part_norm_a(pairs):
                    # reciprocal of the denominators + broadcast DMAs
                    # (HWDGE with a zero-stride partition dim replicates the
                    # single-partition recip row across the 64 head dims).
                    for p in pairs:
                        with nc.allow_low_precision(
                                reason="f16 softmax scale, rel err ~5e-4"):
                            nc.vector.reciprocal(rs[32 * p:32 * p + 16, :],
                                                 ds[32 * p:32 * p + 16, :])
                        nc.sync.dma_start(r0[:, p, :],
                                          rs[32 * p:32 * p + 16, :])
                        for hh in range(2):
                            nc.sync.dma_start(
                                r_bc[64 * hh:64 * (hh + 1), p, :],
                                _part_bcast(r0[hh:hh + 1, p, :], 64))

                def part_norm_b(pairs):
                    for p in pairs:
                        nc.vector.tensor_tensor(o_n[:, p, :], o_pairs[:, p, :],
                                                r_bc[:, p, :], op=OP.mult)

                def part_oproj(chunks, tail=False):
                    for ci, (sc, mcb) in enumerate(chunks):
                        ss = slice(sc * 128, (sc + 1) * 128)
                        ms = slice(mcb * 512, (mcb + 1) * 512)
                        acco = pso.tile([128, 512], F32, tag="acco")
                        for p in range(2):
                            nc.tensor.matmul(acco[:],
                                             o_n[:, p, ss],
                                             wo_t[p][:, ms],
                                             start=(p == 0), stop=(p == 1))
                        outb = op_.tile([128, 512], F16, tag="outb",
                                        name=f"outb_{q4}_{sc}_{mcb}")
                        nc.scalar.copy(outb[:], acco[:])
                        nc.sync.dma_start(
                            partial[qs0 + sc * 128:qs0 + (sc + 1) * 128, ms],
                            outb[:])

                def part_oproj_tail():
                    # the attention stream is over: the sct PSUM banks are
                    # free, so accumulate full [128,1024] rows (2 banks) and
                    # drain with half the instruction count, copies
                    # alternating ACT/DVE
                    for sc in range(4):
                        ss = slice(sc * 128, (sc + 1) * 128)
                        acco2 = psw.tile([128, 2, 512], F32, tag="sct",
                                         name=f"acct_{sc}")
                        for mcb in range(2):
                            ms = slice(mcb * 512, (mcb + 1) * 512)
                            for p in range(2):
                                nc.tensor.matmul(acco2[:, mcb, :],
                                                 o_n[:, p, ss],
                                                 wo_t[p][:, ms],
                                                 start=(p == 0), stop=(p == 1))
                        outb2 = op_.tile([128, 2, 512], F16, tag="outb2",
                                         name=f"outb2_{sc}")
                        if sc % 2 == 1:
                            nc.vector.tensor_copy(outb2[:], acco2[:])
                        else:
                            nc.scalar.copy(outb2[:], acco2[:])
                        nc.sync.dma_start(
                            partial[qs0 + sc * 128:qs0 + (sc + 1) * 128, :],
                            outb2[:])

                return part_norm_a, part_norm_b, part_oproj, part_oproj_tail

            CHUNKS = [(sc, mcb) for sc in range(4) for mcb in range(2)]
            pending = None
            carry = None
            mw = mw0
            for q4 in range(QC):
                # prefetch the NEXT q4's mask during pair 0 (hook 4 for
                # q4>=1; loop-top for q4=0 whose hooks carry the V chunks)
                mw_nxt = None
                if q4 + 1 < QC:
                    mw_nxt = mp.tile([128, KC, 512], F16, tag="mask",
                                     name=f"mw_{q4 + 1}")
                    if q4 == 0:
                        nc.sync.dma_start(mw_nxt[:], mask4[q4 + 1])
                o_collb = np_.tile([65, HPC, 512], F16, tag="ocoll",
                                   name=f"ocoll_{q4}")
                o_pairs = np_.tile([128, 2, 512], F16, tag="opair",
                                   name=f"opairs_{q4}")
                ds = np_.tile([64, 64], F16, tag="ds", name=f"ds_{q4}")

                kb_st = {}

                def _kb(b, half):
                    if half == 0:
                        kb_st[b] = pso.tile([128, 512], F32, tag="acco",
                                            name=f"acck1_{b}")
                    emit_k_block(1, b, kb_st[b], epilogue=(half == 1),
                                 quarters=range(2 * half, 2 * half + 2))

                if pending is not None:
                    pa, pb, po, _pot = pending
                    eqB = {}
                    eq_hooks = ([lambda eqB=eqB, q=q4:
                                 emit_q_half(1, q, eqB, 0, pso, "acco")],
                                [lambda eqB=eqB, q=q4:
                                 emit_q_half(1, q, eqB, 1, pso, "acco")]) \
                        if q4 >= 2 else ([], [])
                    hooks_a = {0: eq_hooks[0],
                               1: eq_hooks[1],
                               3: [lambda pa=pa: pa([0, 1])],
                               4: [lambda mw_nxt=mw_nxt:
                                   nc.sync.dma_start(mw_nxt[:],
                                                     mask4[min(q4 + 1, 3)])
                                   ] if q4 + 1 < QC else [],
                               5: [lambda pb=pb: pb([0, 1])],
                               6: [lambda po=po: po(CHUNKS[0:1])],
                               7: [lambda po=po: po(CHUNKS[1:2])]}
                else:
                    def _vp(sc):
                        emit_v(sc, pso, "acco")

                    k0_st = {}

                    def _k0(b, half):
                        if half == 0:
                            k0_st[b] = pso.tile([128, 512], F32, tag="acco",
                                                name=f"acck0_{b}")
                        emit_k_block(0, b, k0_st[b], epilogue=(half == 1),
                                     quarters=range(2 * half, 2 * half + 2))
                    # NOTE: tiles held across hooks (K/Q accumulators) must
                    # be the LAST "acco" allocation of their first hook and
                    # continue as the FIRST emission of the next hook, so the
                    # 2-buffer rotation never reuses a held bank mid-
                    # accumulation.
                    eq10 = {}
                    hooks_a = {0: [lambda: _vp(0), lambda: _vp(1),
                                   lambda: _k0(1, 0)],
                               1: [lambda: _k0(1, 1),
                                   lambda: _vp(2), lambda: _vp(3)],
                               2: [lambda: _vp(4), lambda: _vp(5),
                                   lambda: _k0(2, 0)],
                               3: [lambda: _k0(2, 1),
                                   lambda: _vp(6), lambda: _vp(7)],
                               4: [lambda: _vp(8), lambda: _vp(9),
                                   lambda: _k0(3, 0)],
                               5: [lambda: _k0(3, 1),
                                   lambda: _vp(10), lambda: _vp(11),
                                   lambda: emit_q_half(1, 0, eq10, 0,
                                                       pso, "acco")],
                               6: [lambda: emit_q_half(1, 0, eq10, 1,
                                                       pso, "acco"),
                                   lambda: _vp(12), lambda: _vp(13),
                                   lambda: _kb(0, 0)],
                               7: [lambda: _kb(0, 1),
                                   lambda: _vp(14), lambda: _vp(15)]}
                accs, pend = emit_pair(q4, 0, mw, hooks_a, carry=carry)

                def _carry0(pend=pend, accs=accs, q4=q4, ocl=o_collb,
                            opr=o_pairs, ds=ds):
                    pend[0](*pend[1])
                    emit_collect(q4, 0, accs, ocl, opr, ds)
                carry = _carry0

                hooks_b = {}
                if pending is not None:
                    pa, pb, po, _pot = pending
                    for i in range(6):
                        hooks_b[i] = [lambda po=po, i=i:
                                      po(CHUNKS[2 + i:3 + i])]
                else:
                    hooks_b = {0: [lambda: _kb(1, 0)],
                               1: [lambda: _kb(1, 1)],
                               2: [lambda: _kb(2, 0)],
                               3: [lambda: _kb(2, 1)],
                               4: [lambda: _kb(3, 0)],
                               5: [lambda: _kb(3, 1)]}
                if q4 + 1 < QC:
                    eqA = {}
                    hooks_b.setdefault(6, []).append(
                        lambda eqA=eqA, q=q4 + 1:
                        emit_q_half(0, q, eqA, 0, pso, "acco"))
                    hooks_b.setdefault(7, []).append(
                        lambda eqA=eqA, q=q4 + 1:
                        emit_q_half(0, q, eqA, 1, pso, "acco"))
                    if q4 == 0:
                        eqC = {}
                        hooks_b.setdefault(3, []).append(
                            lambda eqC=eqC: emit_q_half(1, 1, eqC, 0,
                                                        pso, "acco"))
                        hooks_b.setdefault(5, []).append(
                            lambda eqC=eqC: emit_q_half(1, 1, eqC, 1,
                                                        pso, "acco"))
                if q4 == QC - 1:
                    # normalize pair 0 of the last q4 while pair 1 runs
                    pend_last = make_norm_chain(q4, o_pairs, ds)
                    pa_l, pb_l, po_l, _pot_l = pend_last
                    hooks_b.setdefault(3, []).append(
                        lambda pa_l=pa_l: pa_l([0]))
                    hooks_b.setdefault(6, []).append(
                        lambda pb_l=pb_l: pb_l([0]))
                accs, pend = emit_pair(q4, 1, mw, hooks_b, carry=carry)

                def _carry1(pend=pend, accs=accs, q4=q4, ocl=o_collb,
                            opr=o_pairs, ds=ds):
                    pend[0](*pend[1])
                    emit_collect(q4, 1, accs, ocl, opr, ds)
                carry = _carry1
                if q4 == QC - 1:
                    pending = pend_last
                else:
                    pending = make_norm_chain(q4, o_pairs, ds)
                    mw = mw_nxt

            carry()
            pa_l([1])
            for i in range(48):
                sct_f = psw.tile([128, 2, 512], F32, tag="sct",
                                 name=f"tailwarm_{i}")
                nc.tensor.matmul(sct_f[:, 0, :], warm[0:128, 0:128], warm[:],
                                 start=True, stop=True)
            pa, pb, po, pot = pending
            pb([1])
            pot()

            for ctx in (po_ctx, pa_ctx, ps_ctx, op_ctx, np_ctx, sw_ctx,
                        xp_ctx, mp_ctx):
                ctx.__exit__(None, None, None)

    return nc


_PROGRAM = None


def _get_program():
    global _PROGRAM
    if _PROGRAM is None:
        _PROGRAM = _build_program()
    return _PROGRAM


def _prepare_in_maps(inputs):
    f16 = np.float16
    x = np.asarray(inputs["x"], np.float32)
    mask = np.asarray(inputs["mask"], np.float32)
    Wq = np.asarray(inputs["Wq"], np.float32)
    bq = np.asarray(inputs["bq"], np.float32)
    Wk = np.asarray(inputs["Wk"], np.float32)
    bk = np.asarray(inputs["bk"], np.float32)
    Wv = np.asarray(inputs["Wv"], np.float32)
    Wo = np.asarray(inputs["Wo"], np.float32)

    xT_b = [np.ascontiguousarray(x[b].T).astype(f16) for b in range(B)]
    # mask4[q4, p, c, j] = mask[b,0].T[c*128+p, q4*512+j]
    mask4_b = []
    for b in range(B):
        mt = np.ascontiguousarray(mask[b, 0].T)          # [keys, queries]
        m4 = mt.reshape(KC, 128, QC, 512).transpose(2, 1, 0, 3)
        mask4_b.append(np.ascontiguousarray(m4).astype(f16))

    in_maps = []
    for c in range(NCORES):
        b = c // 4
        h0 = (c % 4) * HPC
        cs = slice(h0 * DK, (h0 + HPC) * DK)
        wq_s = np.ascontiguousarray(Wq[cs, :].T).astype(f16)    # [M, HD]
        wk_s = np.ascontiguousarray(Wk[cs, :].T).astype(f16)
        wv_s = np.ascontiguousarray(Wv[cs, :].T).astype(f16)
        wo_s = np.ascontiguousarray(Wo[:, cs].T).astype(f16)    # [HD, M]
        bq_s = (bq[cs] / 8.0).reshape(2, 128).T.copy().astype(np.float32)
        bk_s = bk[cs].reshape(2, 128).T.copy().astype(np.float32)
        in_maps.append(dict(xT=xT_b[b], mask4=mask4_b[b],
                            wq=wq_s, wk=wk_s, wv=wv_s,
                            wo2=wo_s.reshape(2, 128, M),
                            bq2=bq_s, bk2=bk_s))
    return in_maps


def kernel(x, mask, Wq, bq, Wk, bk, Wv, bv, Wo, bo):
    global LAST_RESULTS
    inputs = dict(x=x, mask=mask, Wq=Wq, bq=bq, Wk=Wk, bk=bk, Wv=Wv, bv=bv,
                  Wo=Wo, bo=bo)
    in_maps = _prepare_in_maps(inputs)

    nc = _get_program()
    res = run_bass_kernel_spmd(nc, in_maps, list(range(NCORES)))
    LAST_RESULTS = res

    out = np.zeros((B, N, M), np.float32)
    for c in range(NCORES):
        out[c // 4] += np.asarray(res.results[c]["partial"], np.float32)
    bv_ = np.asarray(bv, np.float32)
    Wo_ = np.asarray(Wo, np.float32)
    bo_ = np.asarray(bo, np.float32)
    out += (bo_ + bv_ @ Wo_.T)[None, None, :]
    return out
